# revision 10
# baseline (speedup 1.0000x reference)
"""DomainAttention (grouped SE + soft dataset routing) Trainium2 kernel.

Computation (see reference):
  x: (B=4, C=256, D=32, H=64, W=64) f32, split into G=4 depth groups of Dg=8.
  st[b,g,c]   = mean over (Dg,H,W) of x
  h[b,g,n,r]  = relu(st @ w1[n] + b1[n])
  y[b,g,n,c]  = h @ w2[n]^T + b2[n]
  wgt[b,g,n]  = softmax_n(st @ wf[n] + bf[n])
  gate[b,g,c] = sigmoid(sum_n y * wgt)
  out         = x * gate (broadcast over Dg,H,W)

Sharding: 16 independent (b,g) units; 2 per core on 8 cores -> each core
gets the contiguous slice x[b, :, g2*16:(g2+1)*16] of shape (256,16,64,64).
No collectives. Per core: 2 streaming passes over its 67MiB slice
(sum -> tiny SE math on-device -> scale), HBM-roofline bound.
"""

import numpy as np

import concourse.bass as bass
import concourse.tile as tile
from concourse import bacc, mybir
from concourse.bass_utils import run_bass_kernel_spmd

F32 = mybir.dt.float32
AF = mybir.ActivationFunctionType
ALU = mybir.AluOpType

B, C, D, H, W = 4, 256, 32, 64, 64
G = 4
DG = D // G            # 8
SPAT = DG * H * W      # 32768 elements averaged per (b, g, c)
NDS, RED = 3, 16
NR = NDS * RED         # 48
NCORES = 8
CHUNK = 8192
NCHUNK = SPAT // CHUNK  # 4 chunks per (unit, c-half) stream


def _emit(tc, xv, yv, aps, reps=1, loop_n=None):
    """Per-core program. xv/yv: [2 units, 256 c, 32768 spat] DRAM views.

    reps > 1 repeats the whole body back-to-back inside one NEFF; used by
    the timing harness to measure steady-state per-iteration HW time as the
    slope between rep counts (cancels dispatch/transfer overheads).
    """
    nc = tc.nc
    from contextlib import ExitStack

    with ExitStack() as ctx:
        consts = ctx.enter_context(tc.tile_pool(name="consts", bufs=1))
        io = ctx.enter_context(tc.tile_pool(name="io", bufs=5))
        stats = ctx.enter_context(tc.tile_pool(name="stats", bufs=4))
        stp = ctx.enter_context(tc.tile_pool(name="stp", bufs=4))
        gates = ctx.enter_context(tc.tile_pool(name="gates", bufs=4))
        small = ctx.enter_context(tc.tile_pool(name="small", bufs=2))
        psum = ctx.enter_context(tc.tile_pool(name="psum", bufs=2, space="PSUM"))
        psum_y = ctx.enter_context(tc.tile_pool(name="psum_y", bufs=2, space="PSUM"))

        def load_const(name, shape):
            t = consts.tile(list(shape), F32, tag=name)
            nc.sync.dma_start(t, aps[name])
            return t

        wc1_t = load_const("wc1", (128, 2 * NR))   # [c_half, (half, n*r)] / SPAT
        bc1_t = load_const("bc1", (1, NR))
        wc2_t = load_const("wc2", (NR, C))         # [(n,r), c]
        bc2t_t = load_const("bc2t", (128, 2 * NDS))  # b2^T packed per c-half
        wcf_t = load_const("wcf", (128, 2 * NDS))  # wf^T packed per c-half / SPAT
        bcf_t = load_const("bcf", (1, NDS))
        cmask_t = load_const("cmask", (NR, NDS))   # block-diagonal expander
        ones_t = consts.tile([1, 128], F32, tag="ones")
        nc.vector.memset(ones_t, 1.0)

        if loop_n is not None:
            with tc.For_i(0, loop_n, 1):
                _emit_one(tc, nc, xv, yv, io, stats, stp, gates, small, psum,
                          psum_y, wc1_t, bc1_t, wc2_t, bc2t_t, wcf_t, bcf_t,
                          cmask_t, ones_t)
        else:
            for _rep in range(reps):
                _emit_one(tc, nc, xv, yv, io, stats, stp, gates, small, psum,
                          psum_y, wc1_t, bc1_t, wc2_t, bc2t_t, wcf_t, bcf_t,
                          cmask_t, ones_t)


def _emit_one(tc, nc, xv, yv, io, stats, stp, gates, small, psum, psum_y,
              wc1_t, bc1_t, wc2_t, bc2t_t, wcf_t, bcf_t, cmask_t, ones_t):
        gate_tiles = {}
        for u in range(2):
            st_t = {}
            for h in range(2):
                part = stats.tile([128, NCHUNK], F32, tag="part")
                for i in range(NCHUNK):
                    t = io.tile([128, CHUNK], F32, tag="io")
                    nc.sync.dma_start(t, xv[u, h * 128:(h + 1) * 128, bass.ts(i, CHUNK)])
                    nc.vector.reduce_sum(part[:, i:i + 1], t, axis=mybir.AxisListType.X)
                s = stp.tile([128, 1], F32, tag="st")
                nc.vector.reduce_sum(s, part, axis=mybir.AxisListType.X)
                st_t[h] = s

            # h = relu(st @ w1 + b1), transposed into [48, 1] (1/SPAT folded in wc1)
            hp = psum.tile([NR, 1], F32, tag="hp")
            nc.tensor.matmul(hp, wc1_t[:, 0:NR], st_t[0], start=True, stop=False)
            nc.tensor.matmul(hp, wc1_t[:, NR:2 * NR], st_t[1], start=False, stop=False)
            nc.tensor.matmul(hp, bc1_t, ones_t[:, 0:1], start=False, stop=True)
            h_sb = small.tile([NR, 1], F32, tag="h_sb")
            nc.scalar.activation(h_sb, hp, AF.Relu)
            # rhs_y[(n',r), n] = h[n',r] if n'==n else 0
            rhs_y = small.tile([NR, NDS], F32, tag="rhs_y")
            nc.vector.tensor_scalar_mul(rhs_y, cmask_t, h_sb)

            # routing logits + softmax over n (on one partition)
            lg = psum.tile([1, NDS], F32, tag="lg")
            nc.tensor.matmul(lg, st_t[0], wcf_t[:, 0:NDS], start=True, stop=False)
            nc.tensor.matmul(lg, st_t[1], wcf_t[:, NDS:2 * NDS], start=False, stop=False)
            nc.tensor.matmul(lg, ones_t[:, 0:1], bcf_t, start=False, stop=True)
            mx = small.tile([1, 1], F32, tag="mx")
            nc.vector.reduce_max(mx, lg, axis=mybir.AxisListType.X)
            nmx = small.tile([1, 1], F32, tag="nmx")
            nc.scalar.mul(nmx, mx, -1.0)
            e_sb = small.tile([1, NDS], F32, tag="e_sb")
            nc.scalar.activation(e_sb, lg, AF.Exp, bias=nmx)
            ssum = small.tile([1, 1], F32, tag="ssum")
            nc.vector.reduce_sum(ssum, e_sb, axis=mybir.AxisListType.X)
            rs = small.tile([1, 1], F32, tag="rs")
            nc.vector.reciprocal(rs, ssum)
            wgt = small.tile([1, NDS], F32, tag="wgt")
            nc.vector.tensor_scalar_mul(wgt, e_sb, rs)
            # broadcast wgt across 128 partitions via K=1 matmul with ones
            wb = psum_y.tile([128, NDS], F32, tag="wb")
            nc.tensor.matmul(wb, ones_t, wgt, start=True, stop=True)

            for h in range(2):
                yp = psum_y.tile([128, NDS], F32, tag="yp")
                nc.tensor.matmul(yp, wc2_t[:, h * 128:(h + 1) * 128], rhs_y,
                                 start=True, stop=True)
                yb = small.tile([128, NDS], F32, tag="yb")
                nc.vector.tensor_add(yb, yp, bc2t_t[:, h * NDS:(h + 1) * NDS])
                yw = small.tile([128, NDS], F32, tag="yw")
                nc.vector.tensor_mul(yw, yb, wb)
                gp = small.tile([128, 1], F32, tag="gp")
                nc.vector.reduce_sum(gp, yw, axis=mybir.AxisListType.X)
                g_t = gates.tile([128, 1], F32, tag="gate")
                nc.scalar.activation(g_t, gp, AF.Sigmoid)
                gate_tiles[(u, h)] = g_t

        # pass 2: re-stream x, scale by gate, write out
        for u in range(2):
            for h in range(2):
                for i in range(NCHUNK):
                    t = io.tile([128, CHUNK], F32, tag="io")
                    nc.sync.dma_start(t, xv[u, h * 128:(h + 1) * 128, bass.ts(i, CHUNK)])
                    nc.vector.tensor_scalar_mul(t, t, gate_tiles[(u, h)])
                    nc.scalar.dma_start(yv[u, h * 128:(h + 1) * 128, bass.ts(i, CHUNK)], t)


_PROGRAM_CACHE = {}


def _build_program(reps=1, loop_n=None):
    key = (reps, loop_n)
    if key in _PROGRAM_CACHE:
        return _PROGRAM_CACHE[key]
    nc = bacc.Bacc("TRN2", target_bir_lowering=False, debug=False,
                   enable_asserts=False, num_devices=1)
    aps = {}
    xs = nc.dram_tensor("xs", (C, 2 * DG, H, W), F32, kind="ExternalInput").ap()
    for name, shape in [("wc1", (128, 2 * NR)), ("bc1", (1, NR)),
                        ("wc2", (NR, C)), ("bc2t", (128, 2 * NDS)),
                        ("wcf", (128, 2 * NDS)), ("bcf", (1, NDS)),
                        ("cmask", (NR, NDS))]:
        aps[name] = nc.dram_tensor(name, shape, F32, kind="ExternalInput").ap()
    ys = nc.dram_tensor("ys", (C, 2 * DG, H, W), F32, kind="ExternalOutput").ap()

    xv = xs.rearrange("c (u q) hh ww -> u c (q hh ww)", u=2)
    yv = ys.rearrange("c (u q) hh ww -> u c (q hh ww)", u=2)
    with tile.TileContext(nc) as tc:
        _emit(tc, xv, yv, aps, reps=reps, loop_n=loop_n)
    nc.compile()
    _PROGRAM_CACHE[key] = nc
    return nc


def _host_consts(w1, b1, w2, b2, wf, bf):
    inv = 1.0 / SPAT
    w1f = w1.reshape(NR, C)                       # [(n,r), c]
    wc1 = np.concatenate([w1f[:, :128].T, w1f[:, 128:].T], axis=1) * inv
    bc1 = b1.reshape(1, NR)
    wc2 = w2.transpose(0, 2, 1).reshape(NR, C)    # [(n,r), c]
    b2t = b2.T                                    # [c, n]
    bc2t = np.concatenate([b2t[:128, :], b2t[128:, :]], axis=1)
    wcf = np.concatenate([wf[:, :128].T, wf[:, 128:].T], axis=1) * inv
    bcf = bf.reshape(1, NDS)
    cmask = np.kron(np.eye(NDS), np.ones((RED, 1)))  # [48, 3]
    return {k: np.ascontiguousarray(v, dtype=np.float32) for k, v in {
        "wc1": wc1, "bc1": bc1, "wc2": wc2, "bc2t": bc2t,
        "wcf": wcf, "bcf": bcf, "cmask": cmask}.items()}


def make_in_maps(x, w1, b1, w2, b2, wf, bf):
    cs = _host_consts(np.asarray(w1, np.float32), np.asarray(b1, np.float32),
                      np.asarray(w2, np.float32), np.asarray(b2, np.float32),
                      np.asarray(wf, np.float32), np.asarray(bf, np.float32))
    x = np.asarray(x, np.float32)
    in_maps = []
    for k in range(NCORES):
        b, d0 = k // 2, (k % 2) * 2 * DG
        m = dict(cs)
        m["xs"] = np.ascontiguousarray(x[b, :, d0:d0 + 2 * DG])
        in_maps.append(m)
    return in_maps


def gather_output(results):
    out = np.empty((B, C, D, H, W), dtype=np.float32)
    for k in range(NCORES):
        b, d0 = k // 2, (k % 2) * 2 * DG
        out[b, :, d0:d0 + 2 * DG] = results[k]["ys"]
    return out


def kernel(x, w1, b1, w2, b2, wf, bf, _trace=False):
    nc = _build_program()
    in_maps = make_in_maps(x, w1, b1, w2, b2, wf, bf)
    res = run_bass_kernel_spmd(nc, in_maps, core_ids=list(range(NCORES)),
                               trace=_trace)
    out = gather_output(res.results)
    if _trace:
        kernel.last_results = res
    return out
